# revision 6
# baseline (speedup 1.0000x reference)
"""ColorizationNet Trainium2 kernel (8 NeuronCores, SPMD, two phases).

Structure exploited: rows of the big FC input [4096, 32786] share an identical
x_conv prefix (32768 cols), so

    fc_in @ w1.T = x_conv @ w1[:, :32768].T  (one shared matvec, [304])
                 + [pos|chunks] @ w1[:, 32768:].T  ([4096,18] GEMM)

Sharding (core r of 8):
  - conv backbone row-sharded: core r produces the x_conv slice for pooled
    rows [4r, 4r+4) of every channel (halos via zero-padded input windows,
    out-of-image "phantom" rows masked to zero).
  - shared matvec K-sharded to match; the 8 partials [304] are summed on the
    host (collectives under the axon PJRT path measure ~60us+, so the host
    round trip between the two launches is cheaper).
  - phase B: patch FC sharded by patch row, core r handles patches
    [512r, 512(r+1)).

Perf structure (vs the original two-phase version):
  - PE HAM warm-up: dummy matmuls are woven into both phases' tensor queues
    so the conv / matvec / FC matmuls run at 2.4 GHz instead of 1.2.
  - conv pooling is ACT-first: ONE scalar activation (relu+bias+mask) moves
    each conv PSUM to SBUF (relu/bias commute with max), then the 2x2 pool
    runs as two bf16 tensor_max ops split across Vector (vertical pair) and
    GpSimd (horizontal pair), writing the next layer's moving-window tile
    directly.  This removes two scalar ops per conv block from the old
    copy->max->max->act chain.
  - DMA queues: the w1 stream owns the sync/HWDGE queue and is issued first
    (8 chunks so the matvec can chase it); caw/cam ride the scalar queue
    with conv1's stationaries+windows packed FIRST so conv starts ~1us
    earlier.  Phase B splits cbw so the extras GEMM starts before w2/w3 land.
  - xc uses a pooled-column-parity layout so the matvec's first 16 blocks
    only depend on conv3's first row-group.
"""

import sys

for _p in ("/opt/trn_rl_repo",):
    if _p not in sys.path:
        sys.path.insert(0, _p)

import numpy as np
import ml_dtypes
from contextlib import ExitStack

BF16 = ml_dtypes.bfloat16

IMG = 256
CS = 4
G = 64
H1 = 304
H2 = 176
OUT = 48
NCORES = 8

# phase-A packed bf16 consts: [96, 1668], split into two DMAs:
#   block1 (conv1): s1 [18,384]@0 (+ dup at rows 32:50), xs [50,258]@384,
#                   xs2 [34,258]@642   -> cols [0, 900)
#   block2: s2 [80,384]@900, s3 [96,384]@1284 -> cols [900, 1668)
CAW_W = 1668
XS_O = 384
XS2_O = 642
S2_O = 900
S3_O = 1284
# phase-A packed fp32 masks/biases (now 128 rows: both vertical-pair halves):
#   mk1 [128,3] @0, bm1 [128,3] @3, mk2 [128,3] @6, bm2 [128,3] @9, bc3 @12
CAM_W = 13

# phase-B packed bf16 consts: [128, 1440], split into two DMAs:
#   block1: extrasT [18, 512] @0, w1eT [18, 304] @512  -> cols [0, 816)
#   block2: w2a/b [128, 176] @816/@992, w2c [48, 176] @1168,
#           w3a [128, 48] @1344, w3b [48, 48] @1392    -> cols [816, 1440)
CBW_W = 1440
SHCB_W = 6


def _build_s1(c1_w):
    # [18, 3, 128]: rows i = in-row in window; cols m = s*64 + jp*8 + c
    s1 = np.zeros((18, 3, 128), np.float32)
    for dx in range(3):
        for s in range(2):
            for jp in range(8):
                j = 2 * jp + s
                for c in range(8):
                    m = s * 64 + jp * 8 + c
                    for dy in range(3):
                        s1[j + dy, dx, m] = c1_w[c, 0, dy, dx]
    return np.ascontiguousarray(s1.reshape(18, 3 * 128))


def _build_s2(c2_w):
    # [80, 3, 128]: rows k = delta*8 + ci (ci in 0..8); cols m = s*64+jp*16+co
    s2 = np.zeros((80, 3, 128), np.float32)
    for dx in range(3):
        for s in range(2):
            for jp in range(4):
                j2 = 2 * jp + s
                for co in range(16):
                    m = s * 64 + jp * 16 + co
                    for ci in range(8):
                        for dy in range(3):
                            s2[(j2 + dy) * 8 + ci, dx, m] = c2_w[co, ci, dy, dx]
    return np.ascontiguousarray(s2.reshape(80, 3 * 128))


def _build_s3(c3_w):
    # [96, 3, 128]: rows k = delta*16 + ci (ci in 0..16); cols m = s*64+jpp*32+co
    s3 = np.zeros((96, 3, 128), np.float32)
    for dx in range(3):
        for s in range(2):
            for jpp in range(2):
                j3 = 2 * jpp + s
                for co in range(32):
                    m = s * 64 + jpp * 32 + co
                    for ci in range(16):
                        for dy in range(3):
                            s3[(j3 + dy) * 16 + ci, dx, m] = c3_w[co, ci, dy, dx]
    return np.ascontiguousarray(s3.reshape(96, 3 * 128))


def _host_inputs(x, c1_w, c1_b, c2_w, c2_b, c3_w, c3_b, w1, b1, w2, b2, w3, b3):
    """Returns (in_maps_a, in_maps_b_partial, bias0, b1)."""
    x = np.asarray(x, np.float32).reshape(IMG, IMG)
    s1 = _build_s1(np.asarray(c1_w, np.float32))
    s2 = _build_s2(np.asarray(c2_w, np.float32))
    s3 = _build_s3(np.asarray(c3_w, np.float32))

    caw0 = np.zeros((96, CAW_W), np.float32)
    caw0[0:18, 0:384] = s1
    caw0[32:50, 0:384] = s1  # duplicate for the base-32 conv1 window
    caw0[0:80, S2_O : S2_O + 384] = s2
    caw0[0:96, S3_O : S3_O + 384] = s3
    caw0 = caw0.astype(BF16)

    # phase-B packed consts (same for every core except extrasT)
    cbw0 = np.zeros((128, CBW_W), np.float32)
    w1eT = np.asarray(w1, np.float32)[:, 32768:].T  # [18, 304]
    w2T = np.asarray(w2, np.float32).T  # [304, 176]
    w3T = np.asarray(w3, np.float32).T  # [176, 48]
    cbw0[0:18, 512:816] = w1eT
    cbw0[0:128, 816:992] = w2T[0:128]
    cbw0[0:128, 992:1168] = w2T[128:256]
    cbw0[0:48, 1168:1344] = w2T[256:304]
    cbw0[0:128, 1344:1392] = w3T[0:128]
    cbw0[0:48, 1392:1440] = w3T[128:176]
    bias0 = np.zeros((128, 3), np.float32)
    bias0[0:128, 0] = np.asarray(b2, np.float32)[0:128]
    bias0[0:48, 1] = np.asarray(b2, np.float32)[128:176]
    bias0[0:48, 2] = np.asarray(b3, np.float32)

    w1bigT = np.ascontiguousarray(np.asarray(w1, np.float32)[:, :32768].T)  # [32768, 304]
    chunks = x.reshape(G, CS, G, CS).transpose(0, 2, 1, 3).reshape(G * G, CS * CS)
    pi = (np.arange(G * G) // G).astype(np.float32) * CS
    pj = (np.arange(G * G) % G).astype(np.float32) * CS

    # xc2 layout: p2 = 64*e + 32*jpp + co, col b' = 16*g + jj,
    # kglobal = co*1024 + (4r + 2g + jpp)*32 + 2*jj + e
    P2 = np.arange(128)
    co = P2 % 32
    jpp = (P2 // 32) % 2
    e = P2 // 64
    B2 = np.arange(32)
    gg = B2 // 16
    jj = B2 % 16

    c1b = np.asarray(c1_b, np.float32)
    c2b = np.asarray(c2_b, np.float32)
    c3b = np.asarray(c3_b, np.float32)

    maps_a, maps_b = [], []
    for r in range(NCORES):
        # xs: x rows [32r-7, 32r+43), cols padded by 1 each side
        xs = np.zeros((50, 258), np.float32)
        lo = 32 * r - 7
        hi = 32 * r + 43
        slo, shi = max(lo, 0), min(hi, IMG)
        xs[slo - lo : shi - lo, 1:257] = x[slo:shi, :]
        xs = xs.astype(BF16)
        caw = caw0.copy()
        caw[0:50, XS_O : XS_O + 258] = xs
        caw[0:34, XS2_O : XS2_O + 258] = xs[16:50]

        cam = np.zeros((128, CAM_W), np.float32)
        # row-validity masks (zero out-of-image "phantom" pooled rows);
        # duplicated on partitions 64:128 (the s=1 vertical-pair half)
        for b in range(3):
            for jp in range(8):
                valid = 0 <= (16 * r - 3 + 8 * b + jp) < 128
                for s in range(2):
                    o = 64 * s
                    cam[o + jp * 8 : o + jp * 8 + 8, 0 + b] = 1.0 if valid else 0.0
                    cam[o + jp * 8 : o + jp * 8 + 8, 3 + b] = c1b if valid else 0.0
            for jp in range(4):
                valid = 0 <= (8 * r - 1 + 4 * b + jp) < 64
                for s in range(2):
                    o = 64 * s
                    cam[o + jp * 16 : o + jp * 16 + 16, 6 + b] = 1.0 if valid else 0.0
                    cam[o + jp * 16 : o + jp * 16 + 16, 9 + b] = c2b if valid else 0.0
        cam[0:128, 12] = np.tile(c3b, 4)

        # w1ps2 [128, 32*304] matching the xc2 layout
        kg = (
            co[None, :] * 1024
            + (4 * r + 2 * gg[:, None] + jpp[None, :]) * 32
            + 2 * jj[:, None]
            + e[None, :]
        )  # [32, 128]
        w1ps = np.ascontiguousarray(
            w1bigT[kg.ravel()].reshape(32, 128, 304).transpose(1, 0, 2).reshape(128, 32 * 304)
        ).astype(BF16)
        maps_a.append({"caw": caw, "cam": cam, "w1ps": w1ps})

        cbw = cbw0.copy()
        sl = slice(512 * r, 512 * (r + 1))
        cbw[0, 0:512] = pi[sl]
        cbw[1, 0:512] = pj[sl]
        cbw[2:18, 0:512] = chunks[sl].T
        maps_b.append({"cbw": cbw.astype(BF16)})
    return maps_a, maps_b, bias0, np.asarray(b1, np.float32)


def _mk_nc():
    import concourse.bacc as bacc

    return bacc.Bacc("TRN2", target_bir_lowering=False, debug=False, num_devices=NCORES)


def _build_phase_a():
    """Convs + sharded shared-matvec partial. Output: part [1, 304]."""
    import concourse.tile as tile
    from concourse import mybir

    f32 = mybir.dt.float32
    bf16 = mybir.dt.bfloat16
    AF = mybir.ActivationFunctionType
    nc = _mk_nc()

    def din(name, shape, dt=f32):
        return nc.dram_tensor(name, list(shape), dt, kind="ExternalInput").ap()

    caw_d = din("caw", (96, CAW_W), bf16)
    cam_d = din("cam", (128, CAM_W), f32)
    w1ps_d = din("w1ps", (128, 32 * 304), bf16)
    part_d = nc.dram_tensor("part", [1, 304], f32, kind="ExternalOutput").ap()

    with tile.TileContext(nc) as tc, ExitStack() as ctx:
        cpool = ctx.enter_context(tc.tile_pool(name="consts", bufs=1))
        apool = ctx.enter_context(tc.tile_pool(name="act", bufs=3))
        vpool = ctx.enter_context(tc.tile_pool(name="vmax", bufs=3))
        pconv = ctx.enter_context(tc.tile_pool(name="pconv", bufs=3, space="PSUM"))
        pdum = ctx.enter_context(tc.tile_pool(name="pdum", bufs=1, space="PSUM"))
        pmv = ctx.enter_context(tc.tile_pool(name="pmv", bufs=1, space="PSUM"))

        # ---- DMAs first (sync queue: the w1 stream in 8 chunks; scalar
        # queue: caw block1 (conv1 needs), caw block2, cam), THEN the act
        # table / PE warm-ups so they don't delay DMA issue.
        wst = cpool.tile([128, 32 * 304], bf16, tag="w1s")
        CH = 8
        chw = 32 * 304 // CH
        for c in range(CH):
            nc.sync.dma_start(wst[:, c * chw : (c + 1) * chw], w1ps_d[:, c * chw : (c + 1) * chw])

        caw_t = cpool.tile([96, CAW_W], bf16, tag="caw")
        nc.scalar.dma_start(caw_t[:, 0:S2_O], caw_d[:, 0:S2_O])
        nc.scalar.dma_start(caw_t[:, S2_O:CAW_W], caw_d[:, S2_O:CAW_W])
        cam_t = cpool.tile([128, CAM_W], f32, tag="cam")
        nc.scalar.dma_start(cam_t[:], cam_d)

        # ---- engine warm-up: act table + PE dummy-matmul scratch
        scr = cpool.tile([1, 1], f32, tag="scr")
        nc.vector.memset(scr[:], 0.0)
        scr2 = cpool.tile([1, 1], f32, tag="scr2")
        nc.scalar.activation(scr2[:], scr[:], AF.Relu)

        dum = cpool.tile([128, 256], bf16, tag="dum")
        nc.vector.memset(dum[:], 0.0)
        psd = pdum.tile([128, 256], f32, tag="psd")

        def dummy(n):
            for _ in range(n):
                nc.tensor.matmul(psd[:], lhsT=dum[:, 0:128], rhs=dum[:], start=True, stop=True)

        def s1ap(dx, base):
            return caw_t[base : base + 18, 128 * dx : 128 * (dx + 1)]

        def s2ap(dx):
            return caw_t[0:80, S2_O + 128 * dx : S2_O + 128 * (dx + 1)]

        def s3ap(dx):
            return caw_t[0:96, S3_O + 128 * dx : S3_O + 128 * (dx + 1)]

        mk1 = lambda b: cam_t[0:64, 0 + b : 1 + b]
        bm1 = lambda b: cam_t[0:64, 3 + b : 4 + b]
        mk2 = lambda b: cam_t[0:64, 6 + b : 7 + b]
        bm2 = lambda b: cam_t[0:64, 9 + b : 10 + b]
        bc3 = cam_t[0:128, 12:13]

        # next-layer moving-window tiles (fused: all windows in one tile)
        m2 = cpool.tile([80, 3 * 130], bf16, tag="m2", name="m2")
        m3 = cpool.tile([96, 2 * 66], bf16, tag="m3", name="m3")
        xc = cpool.tile([128, 32], bf16, tag="xc", name="xc")
        nc.vector.memset(m2[:], 0.0)
        nc.vector.memset(m3[:], 0.0)

        dummy(6)  # head warm-up (runs while caw lands)

        # ---- conv1: 3 blocks of 16 output rows -> m2 windows
        # chain per block: 3 MM -> vector copy(top half psum->sbuf) ->
        #   vector vertical-pair max (PSUM x SBUF) -> gpsimd horizontal-pair
        #   max -> scalar ACT(relu+bias+mask) -> m2 window; tails are gpsimd
        #   copies of the finished window rows.
        win1 = [(0, XS_O, 0), (0, XS2_O, 0), (32, XS_O, 32)]
        for b in range(3):
            rlo, colo, base = win1[b]
            ps = pconv.tile([128, 256], f32, tag="cps")
            for dx in range(3):
                nc.tensor.matmul(
                    ps[:],
                    lhsT=s1ap(dx, base),
                    rhs=caw_t[rlo : rlo + 18, colo + dx : colo + dx + 256],
                    start=(dx == 0),
                    stop=(dx == 2),
                )
            dummy(2)
            vt = apool.tile([64, 256], f32, tag="vt1")
            nc.vector.tensor_copy(vt[:], ps[0:64, :])
            v = vpool.tile([64, 256], bf16, tag="v1")
            nc.vector.tensor_max(v[:], ps[64:128, :], vt[:])
            vv = v[:].rearrange("p (x t) -> p x t", t=2)
            ph = vpool.tile([64, 128], bf16, tag="ph1")
            nc.vector.tensor_max(ph[:], vv[:, :, 0], vv[:, :, 1])
            nc.scalar.activation(
                m2[0:64, 130 * b + 1 : 130 * b + 129], ph[:], AF.Relu,
                bias=bm1(b), scale=mk1(b),
            )
            if b >= 1:  # rows 8b, 8b+1 also tail rows 8..10 of previous window
                nc.scalar.copy(
                    m2[64:80, 130 * (b - 1) + 1 : 130 * (b - 1) + 129],
                    m2[0:16, 130 * b + 1 : 130 * b + 129],
                )

        # ---- conv2: 3 blocks of 8 output rows -> m3 windows
        for b in range(3):
            ps = pconv.tile([128, 128], f32, tag="cps")
            for dx in range(3):
                nc.tensor.matmul(
                    ps[:],
                    lhsT=s2ap(dx),
                    rhs=m2[:, 130 * b + dx : 130 * b + dx + 128],
                    start=(dx == 0),
                    stop=(dx == 2),
                )
            dummy(2)
            vt = apool.tile([64, 128], f32, tag="vt2")
            nc.vector.tensor_copy(vt[:], ps[0:64, :])
            v = vpool.tile([64, 128], bf16, tag="v2")
            nc.vector.tensor_max(v[:], ps[64:128, :], vt[:])
            vv = v[:].rearrange("p (x t) -> p x t", t=2)
            ph = vpool.tile([64, 64], bf16, tag="ph2")
            nc.vector.tensor_max(ph[:], vv[:, :, 0], vv[:, :, 1])
            if b == 0:
                nc.scalar.activation(
                    m3[0:64, 1:65], ph[:], AF.Relu, bias=bm2(0), scale=mk2(0)
                )
            elif b == 1:
                nc.scalar.activation(
                    m3[0:64, 67:131], ph[:], AF.Relu, bias=bm2(1), scale=mk2(1)
                )
                nc.scalar.copy(m3[64:96, 1:65], m3[0:32, 67:131])
            else:
                nc.scalar.activation(
                    m3[64:96, 67:131], ph[0:32, :], AF.Relu,
                    bias=cam_t[0:32, 11:12], scale=cam_t[0:32, 8:9],
                )

        # ---- conv3: 2 row-groups of 4 output rows -> xc [128, 32]
        # xc layout: col b' = 16g + jj, partition 64e + q  (e = pooled-col parity)
        xcr = cpool.tile([128, 32], bf16, tag="xcr", name="xcr")
        for g in range(2):
            ps = pconv.tile([128, 64], f32, tag="cps")
            for dx in range(3):
                nc.tensor.matmul(
                    ps[:],
                    lhsT=s3ap(dx),
                    rhs=m3[:, 66 * g + dx : 66 * g + dx + 64],
                    start=(dx == 0),
                    stop=(dx == 2),
                )
            dummy(2)
            vt = apool.tile([64, 64], f32, tag="vt3")
            nc.vector.tensor_copy(vt[:], ps[0:64, :])
            v = vpool.tile([64, 64], bf16, tag="v3")
            nc.vector.tensor_max(v[:], ps[64:128, :], vt[:])
            vv = v[:].rearrange("p (x t) -> p x t", t=4)
            nc.vector.tensor_max(
                xcr[0:64, 16 * g : 16 * g + 16], vv[:, :, 0], vv[:, :, 1]
            )
            nc.vector.tensor_max(
                xcr[64:128, 16 * g : 16 * g + 16], vv[:, :, 2], vv[:, :, 3]
            )
            nc.scalar.activation(
                xc[:, 16 * g : 16 * g + 16], xcr[:, 16 * g : 16 * g + 16],
                AF.Relu, bias=bc3,
            )
            if g == 0:
                dummy(4)

        # ---- shared matvec partial [1, 304]
        ps_mv = pmv.tile([1, 304], f32, tag="mv")
        for b in range(32):
            nc.tensor.matmul(
                ps_mv[:],
                lhsT=xc[:, b : b + 1],
                rhs=wst[:, 304 * b : 304 * (b + 1)],
                start=(b == 0),
                stop=(b == 31),
            )
        part_s = cpool.tile([1, 304], f32, tag="part")
        nc.scalar.copy(part_s[:], ps_mv[:])
        nc.sync.dma_start(part_d, part_s[:])

    nc.compile()
    return nc


def _build_phase_b():
    """Patch FC for this core's 512 patches, given summed shared vector."""
    import concourse.tile as tile
    from concourse import mybir

    f32 = mybir.dt.float32
    bf16 = mybir.dt.bfloat16
    AF = mybir.ActivationFunctionType
    nc = _mk_nc()

    cbw_d = nc.dram_tensor("cbw", [128, CBW_W], bf16, kind="ExternalInput").ap()
    shcb_d = nc.dram_tensor("shcb", [128, SHCB_W], f32, kind="ExternalInput").ap()
    yout_d = nc.dram_tensor("yout", [48, 512], f32, kind="ExternalOutput").ap()

    mblk = [(0, 128), (128, 128), (256, 48)]
    qblk = [(0, 128), (128, 48)]

    with tile.TileContext(nc) as tc, ExitStack() as ctx:
        cpool = ctx.enter_context(tc.tile_pool(name="consts", bufs=1))
        fpool = ctx.enter_context(tc.tile_pool(name="fc", bufs=1))
        pfc = ctx.enter_context(tc.tile_pool(name="pfc", bufs=1, space="PSUM"))
        phh = ctx.enter_context(tc.tile_pool(name="phh", bufs=3, space="PSUM"))
        pdum = ctx.enter_context(tc.tile_pool(name="pdum", bufs=1, space="PSUM"))

        # DMAs first, then act tables / PE warm-up
        cbw = cpool.tile([128, CBW_W], bf16, tag="cbw")
        nc.sync.dma_start(cbw[:, 0:816], cbw_d[:, 0:816])
        nc.sync.dma_start(cbw[:, 816:CBW_W], cbw_d[:, 816:CBW_W])
        shcb = cpool.tile([128, SHCB_W], f32, tag="shcb")
        nc.scalar.dma_start(shcb[:], shcb_d)

        scr = cpool.tile([1, 1], f32, tag="scr")
        nc.vector.memset(scr[:], 0.0)
        scr2 = cpool.tile([1, 1], f32, tag="scr2")
        nc.scalar.activation(scr2[:], scr[:], AF.Relu)
        nc.scalar.activation(scr2[:], scr[:], AF.Sigmoid)

        dum = cpool.tile([128, 256], bf16, tag="dum")
        nc.vector.memset(dum[:], 0.0)
        psd = pdum.tile([128, 256], f32, tag="psd")

        def dummy(n):
            for _ in range(n):
                nc.tensor.matmul(psd[:], lhsT=dum[:, 0:128], rhs=dum[:], start=True, stop=True)

        extrasT = cbw[0:18, 0:512]
        w1eT = cbw[0:18, 512:816]
        w2T_t = [cbw[0:128, 816:992], cbw[0:128, 992:1168], cbw[0:48, 1168:1344]]
        w3T_t = [cbw[0:128, 1344:1392], cbw[0:48, 1392:1440]]
        b2c_t = [shcb[0:128, 3:4], shcb[0:48, 4:5]]
        b3c_t = shcb[0:48, 5:6]
        sh_t = [shcb[0:128, 0:1], shcb[0:128, 1:2], shcb[0:48, 2:3]]

        dummy(5)  # head warm-up (runs while cbw lands)

        h1_t = []
        for i, (off, mb) in enumerate(mblk):
            ps_e = pfc.tile([mb, 512], f32, tag=f"pse{i}")
            nc.tensor.matmul(
                ps_e[:],
                lhsT=w1eT[:, off : off + mb],
                rhs=extrasT,
                start=True,
                stop=True,
            )
            h1 = fpool.tile([mb, 512], bf16, tag=f"h1{i}")
            from concourse import mybir as _mb
            nc.vector.tensor_scalar(h1[:], ps_e[:], sh_t[i], 0.0, _mb.AluOpType.add, _mb.AluOpType.max)
            h1_t.append(h1)
        dummy(6)

        h2_t = []
        for q, (qoff, mq) in enumerate(qblk):
            ps_h = phh.tile([mq, 512], f32, tag="psh")
            for i, (off, mb) in enumerate(mblk):
                nc.tensor.matmul(
                    ps_h[:],
                    lhsT=w2T_t[i][:, qoff : qoff + mq],
                    rhs=h1_t[i][:],
                    start=(i == 0),
                    stop=(i == 2),
                )
            h2 = fpool.tile([mq, 512], bf16, tag=f"h2{q}")
            nc.scalar.activation(h2[:], ps_h[:], AF.Relu, bias=b2c_t[q])
            h2_t.append(h2)

        ps_o = phh.tile([48, 512], f32, tag="psh")
        for q, (qoff, mq) in enumerate(qblk):
            nc.tensor.matmul(
                ps_o[:],
                lhsT=w3T_t[q],
                rhs=h2_t[q][:],
                start=(q == 0),
                stop=(q == 1),
            )
        outs = fpool.tile([48, 512], f32, tag="outs")
        nc.scalar.activation(outs[:], ps_o[:], AF.Sigmoid, bias=b3c_t)
        nc.sync.dma_start(yout_d, outs[:])

    nc.compile()
    return nc


def _shcb_pack(sh, bias0):
    shcb = np.zeros((128, SHCB_W), np.float32)
    shcb[0:128, 0] = sh[0:128]
    shcb[0:128, 1] = sh[128:256]
    shcb[0:48, 2] = sh[256:304]
    shcb[:, 3:6] = bias0
    return shcb


def _run(maps_a, maps_b, bias0, b1, trace=False, trace_cores=None):
    from concourse.bass_utils import run_bass_kernel_spmd

    nca = _build_phase_a()
    res_a = run_bass_kernel_spmd(
        nca, maps_a, list(range(NCORES)), trace=trace, trace_cores=trace_cores
    )
    sh = np.sum([res_a.results[r]["part"][0] for r in range(NCORES)], axis=0) + b1
    shcb = _shcb_pack(sh, bias0)
    for mb in maps_b:
        mb["shcb"] = shcb
    ncb = _build_phase_b()
    res_b = run_bass_kernel_spmd(
        ncb, maps_b, list(range(NCORES)), trace=trace, trace_cores=trace_cores
    )
    full = np.empty((G * G, OUT), np.float32)
    for r in range(NCORES):
        full[512 * r : 512 * (r + 1), :] = res_b.results[r]["yout"].T
    return full.reshape(3, IMG, IMG), res_a, res_b


def kernel(**inputs):
    maps_a, maps_b, bias0, b1 = _host_inputs(**inputs)
    out, _, _ = _run(maps_a, maps_b, bias0, b1)
    return out


if __name__ == "__main__":
    import reference

    inp = {k: np.asarray(v) for k, v in reference.setup_inputs().items()}
    got = kernel(**inp)
    exp = np.asarray(reference.reference(**reference.setup_inputs()))
    err = np.abs(got - exp).max() / max(np.abs(exp).max(), 1e-9)
    print("Relative error:", err)
